# revision 8
# baseline (speedup 1.0000x reference)
"""Single-head causal self-attention on 8 Trainium2 NeuronCores (Bass/Tile).

Problem: x [1024, 256, 384], Wq/Wk/Wv [384, 64] ->
  q,k,v = x@W;  wei = softmax(mask(q k^T / sqrt(384)));  out = wei @ v
Output: [1024, 256, 64] fp32.

Strategy (data-parallel over batch, 128 batches per core, all-bf16 matmuls):
  - Host pre-transposes x to bf16 xt4[g, p, c, j] = x[4g + j//256, j%256,
    128c+p] (groups of 4 batches = 2 pairs) so the contraction dim C=384
    lands on SBUF partitions with contiguous 6KB DMA rows, and input DMA
    bytes are halved vs fp32.
  - Per batch pair (2 batches, shared instructions where possible):
      ps_qk [128,1024->512] = [Wk|Wq]^T x^T   (3 mm, N=512; k rows 0:64)
      k_sb/q_sb [64,512] bf16 <- PSUM copies  (DVE unshifted / ACT shifted)
      ps_v  [128,4,64] = x-stationary v-proj  (12 mm, N=64 -> v in [t,h])
      vaug  [128,2,66] bf16, ones at col 64   (one strided copy per batch)
      psw   [128,384]  = weiT[s0,t 0:256 | s1,t1]  (2 mm per batch, K=64)
      P     [128,384] bf16 = exp(psw/sqrt(384))    (one ACT op per batch)
      mask  diag blocks 0 and 2 of P viewed [128,3,128] (one DVE mul)
      pso   [128,4,65] out[t,h]+denom: lhsT=P-block stationary, rhs=vaug
             (3 mm per batch, N=65; col 64 = softmax denominator per t)
      recip [128,4] = 1/denom (per-partition -> cheap DVE reciprocal)
      out   = pso[:, :, 0:64] * recip (broadcast along h)
  - Issue order is software-pipelined one pair ahead so the PE queue never
    drains (p-state ramps to full clock).
  - Output written as [g, p, 8, 64] blocks; host reassembles [b, t, h].
"""

from contextlib import ExitStack

import numpy as np
import ml_dtypes

import concourse.bass as bass
import concourse.bacc as bacc
import concourse.tile as tile
from concourse import mybir
from concourse.bass_utils import run_bass_kernel_spmd

N_CORES = 8
B = 1024
T = 256
C = 384
H = 64
BPC = B // N_CORES  # 128 batches per core
NCHUNK = C // 128  # 3
NPAIR = BPC // 2  # 64 pairs per core
NGROUP = BPC // 4  # 32 groups (2 pairs) per core
SCALE = float(C) ** -0.5

F32 = mybir.dt.float32
BF16 = mybir.dt.bfloat16
BF = ml_dtypes.bfloat16


def build_nc(bpc: int = BPC):
    npair = bpc // 2
    ngroup = bpc // 4
    nc = bacc.Bacc(
        "TRN2", target_bir_lowering=False, debug=False, num_devices=N_CORES
    )

    xt4 = nc.dram_tensor("xt4", [ngroup, 128, NCHUNK, 1024], BF16, kind="ExternalInput").ap()
    wkq = nc.dram_tensor("wkq", [128, NCHUNK, 128], BF16, kind="ExternalInput").ap()
    wv = nc.dram_tensor("wv", [128, NCHUNK, H], BF16, kind="ExternalInput").ap()
    mask = nc.dram_tensor("mask", [128, 2, 128], BF16, kind="ExternalInput").ap()
    outF = nc.dram_tensor("outF", [ngroup, 128, 8, H], F32, kind="ExternalOutput").ap()

    with ExitStack() as ctx:
        tc = ctx.enter_context(tile.TileContext(nc))

        const = ctx.enter_context(tc.tile_pool(name="const", bufs=1))
        wkq_sb = const.tile([128, NCHUNK, 128], BF16, tag="wkq")
        nc.sync.dma_start(wkq_sb[:], wkq)
        wv_sb = const.tile([128, NCHUNK, H], BF16, tag="wv")
        nc.sync.dma_start(wv_sb[:], wv)
        mask_sb = const.tile([128, 2, 128], BF16, tag="mask")
        nc.sync.dma_start(mask_sb[:], mask)

        # Persistent v_aug tiles: [s, h] halves at [:, i, 0:64], ones col at
        # [:, i, 64] (softmax denominator trick). 4 slots = 2 per pair x 2
        # pipelined pairs.
        NSLOT = 8
        vaug = []
        for i in range(NSLOT):
            v_t = const.tile([128, 2, 66], BF16, tag=f"vaug{i}")
            nc.gpsimd.memset(v_t[:, 0, 64:65], 1.0)
            nc.gpsimd.memset(v_t[:, 1, 64:65], 1.0)
            vaug.append(v_t)

        xt_pool = ctx.enter_context(tc.tile_pool(name="xt", bufs=3))
        qk_pool = ctx.enter_context(tc.tile_pool(name="qk", bufs=2))
        p_pool = ctx.enter_context(tc.tile_pool(name="pp", bufs=6))
        o_pool = ctx.enter_context(tc.tile_pool(name="oo", bufs=2))
        r_pool = ctx.enter_context(tc.tile_pool(name="rr", bufs=2))
        psqk_pool = ctx.enter_context(tc.tile_pool(name="psqk", bufs=2, space="PSUM"))
        psv_pool = ctx.enter_context(tc.tile_pool(name="psv", bufs=2, space="PSUM"))
        psw_pool = ctx.enter_context(tc.tile_pool(name="psw", bufs=2, space="PSUM"))
        pso_pool = ctx.enter_context(tc.tile_pool(name="pso", bufs=2, space="PSUM"))

        xt_tiles = {}
        stage = {}  # pair -> (pso, vaug slots) for the lagged epilogue

        def front(p):
            """DMA + projections + wei/exp/mask for pair p."""
            g, r = divmod(p, 2)
            if r == 0:
                xt = xt_pool.tile([128, NCHUNK, 1024], BF16, tag="xt")
                nc.sync.dma_start(xt[:], xt4[g])
                xt_tiles[g] = xt
            xt = xt_tiles[g]
            base = 512 * r

            ps_qk = psqk_pool.tile([128, 512], F32, tag="psqk")
            for c in range(NCHUNK):
                nc.tensor.matmul(
                    ps_qk[:],
                    lhsT=wkq_sb[:, c, :],
                    rhs=xt[:, c, base : base + 512],
                    start=(c == 0),
                    stop=(c == NCHUNK - 1),
                )

            k_sb = qk_pool.tile([H, 512], BF16, tag="k")
            nc.vector.tensor_copy(k_sb[:], ps_qk[0:H, :])
            q_sb = qk_pool.tile([H, 512], BF16, tag="q")
            nc.scalar.copy(q_sb[:], ps_qk[H:128, :])

            # v[t, h] via x-stationary projection: 4 t-blocks x 3 chunks.
            ps_v = psv_pool.tile([128, 4, H], F32, tag="psv")
            for blk in range(4):
                for c in range(NCHUNK):
                    nc.tensor.matmul(
                        ps_v[:, blk, :],
                        lhsT=xt[:, c, base + 128 * blk : base + 128 * (blk + 1)],
                        rhs=wv_sb[:, c, :],
                        start=(c == 0),
                        stop=(c == NCHUNK - 1),
                    )

            slots = (vaug[(2 * p) % NSLOT], vaug[(2 * p + 1) % NSLOT])
            psws = []
            for j in range(2):
                nc.vector.tensor_copy(
                    slots[j][:, :, 0:64], ps_v[:, 2 * j : 2 * j + 2, :]
                )
                psw = psw_pool.tile([128, 384], F32, tag="psw")
                nc.tensor.matmul(
                    psw[:, 0:256],
                    lhsT=k_sb[:, 256 * j : 256 * j + 128],
                    rhs=q_sb[:, 256 * j : 256 * j + 256],
                    start=True,
                    stop=True,
                )
                nc.tensor.matmul(
                    psw[:, 256:384],
                    lhsT=k_sb[:, 256 * j + 128 : 256 * j + 256],
                    rhs=q_sb[:, 256 * j + 128 : 256 * j + 256],
                    start=True,
                    stop=True,
                )
                psws.append(psw)

            ps = []
            for j in range(2):
                P = p_pool.tile([128, 3, 128], BF16, tag="p")
                nc.scalar.activation(
                    P[:].rearrange("p a b -> p (a b)"),
                    psws[j][:],
                    mybir.ActivationFunctionType.Exp,
                    scale=SCALE,
                )
                nc.gpsimd.tensor_mul(P[:, 0::2, :], P[:, 0::2, :], mask_sb[:])
                ps.append(P)

            stage[p] = (ps, slots)

        def back(p):
            """Out matmuls + normalize + output DMA for pair p."""
            g, r = divmod(p, 2)
            ps, slots = stage.pop(p)
            pso = pso_pool.tile([128, 4, 65], F32, tag="pso")
            for j in range(2):
                P, va = ps[j], slots[j]
                nc.tensor.matmul(
                    pso[:, 2 * j, :],
                    lhsT=P[:, 0, :],
                    rhs=va[:, 0, 0:65],
                    start=True,
                    stop=True,
                )
                nc.tensor.matmul(
                    pso[:, 2 * j + 1, :],
                    lhsT=P[:, 1, :],
                    rhs=va[:, 0, 0:65],
                    start=True,
                    stop=False,
                )
                nc.tensor.matmul(
                    pso[:, 2 * j + 1, :],
                    lhsT=P[:, 2, :],
                    rhs=va[:, 1, 0:65],
                    start=False,
                    stop=True,
                )

            recip = r_pool.tile([128, 4], F32, tag="recip")
            nc.vector.reciprocal(recip[:], pso[:, :, 64])

            if r == 0:
                out_sb = o_pool.tile([128, 8, H], F32, tag="out")
                stage[("o", g)] = out_sb
            else:
                out_sb = stage[("o", g)]
            nc.vector.tensor_mul(
                out_sb[:, 4 * r : 4 * r + 4, :],
                pso[:, :, 0:64],
                recip[:].unsqueeze(-1).broadcast_to([128, 4, H]),
            )
            if r == 1:
                nc.gpsimd.dma_start(outF[g], stage.pop(("o", g))[:])

        # Software pipeline: front(p) runs two pairs ahead of back(p-2) so the
        # PE always has independent work queued while the exp->mask chain
        # settles.
        LAG = 2
        for p in range(LAG):
            front(p)
        for p in range(LAG, npair):
            front(p)
            back(p - LAG)
        for p in range(npair - LAG, npair):
            back(p)

    nc.finalize()
    return nc


def _host_inputs(x, Wq, Wk, Wv):
    B_, T_, C_ = x.shape
    assert (B_, T_, C_) == (B, T, C), (B_, T_, C_)
    # xt4[g, p, c, j] = x[4g + j//256, j%256, 128c + p], bf16
    xh = np.ascontiguousarray(
        x.reshape(B // 4, 4, T, NCHUNK, 128).transpose(0, 4, 3, 1, 2)
        .reshape(B // 4, 128, NCHUNK, 4 * T)
    ).astype(BF)
    wkq_h = np.ascontiguousarray(
        np.concatenate([Wk, Wq], axis=1).reshape(NCHUNK, 128, 128).transpose(1, 0, 2)
    ).astype(BF)
    wv_h = np.ascontiguousarray(
        Wv.reshape(NCHUNK, 128, H).transpose(1, 0, 2)
    ).astype(BF)
    mask1 = np.triu(np.ones((128, 128), dtype=np.float32))
    mask_h = np.ascontiguousarray(np.stack([mask1, mask1], axis=1)).astype(BF)
    return xh, wkq_h, wv_h, mask_h


def _host_output(res, bpc=BPC):
    # outF [ngroup, 128, 8, 64]: block 2j+k = batch j of group, t-half k.
    outs = []
    for i in range(N_CORES):
        a = res.results[i]["outF"]  # [32, 128, 8, 64]
        a = a.reshape(bpc // 4, 128, 4, 2, H).transpose(0, 2, 3, 1, 4)
        outs.append(a.reshape(bpc, T, H))
    return np.ascontiguousarray(np.concatenate(outs, axis=0))


def kernel(x, Wq, Wk, Wv):
    x = np.asarray(x, dtype=np.float32)
    Wq = np.asarray(Wq, dtype=np.float32)
    Wk = np.asarray(Wk, dtype=np.float32)
    Wv = np.asarray(Wv, dtype=np.float32)

    xh, wkq_h, wv_h, mask_h = _host_inputs(x, Wq, Wk, Wv)

    nc = build_nc(BPC)
    in_maps = [
        {
            "xt4": xh[i * NGROUP : (i + 1) * NGROUP],
            "wkq": wkq_h,
            "wv": wv_h,
            "mask": mask_h,
        }
        for i in range(N_CORES)
    ]
    res = run_bass_kernel_spmd(nc, in_maps, list(range(N_CORES)))
    return _host_output(res, BPC)
